# revision 9
# baseline (speedup 1.0000x reference)
"""ChemProp message-to-node + MLP kernel for 8 TRN2 NeuronCores.

Strategy (no collectives needed):
  - Host sorts edges by destination node; core c owns nodes
    [c*NPC, (c+1)*NPC) and receives exactly the edge features destined
    for its nodes, pre-permuted into a "degree-slot" layout so the
    device-side segment-sum is pure contiguous streaming adds.
  - Node groups of <=512 (one PSUM window). Within a group, nodes are
    sorted by degree (desc). Slot d holds the d-th edge of every node
    with degree > d, so each slot is a contiguous run of columns that
    adds elementwise into a prefix of the group's message accumulator.
  - Layout is feature-major ([256, cols] split into 2x128 partitions) so
    the MLP runs without any transposes: hidden^T = W1^T @ cat^T etc.
  - Segment-sum accumulates in PSUM via identity matmuls (TensorE), the
    MLP runs in bf16 with f32 PSUM accumulation.
  - Per-core output slice is returned feature-major; host transposes,
    un-permutes and concatenates.
"""

import os

import numpy as np
import ml_dtypes

import concourse.bacc as bacc
import concourse.mybir as mybir
import concourse.tile as tile
from concourse.bass_utils import run_bass_kernel_spmd
from concourse.masks import make_identity

LAST_EXEC_TIME_NS = None
LAST_TRACE_DIR = None

NC = 8          # cores
GRP = 512       # nodes per group (one PSUM window)
CHUNK = 4096    # stream-chunk columns
STREAM_BF16 = True   # v2: bf16 edge stream + TensorE adds; False: f32 + DVE

BF16 = mybir.dt.bfloat16
F32 = mybir.dt.float32
NP_BF16 = ml_dtypes.bfloat16


# ----------------------------------------------------------------- host side
def _preprocess(r, h, nbrs):
    """Build per-core streams/permutations. Returns layout + per-core arrays."""
    n_nodes, Fdim = r.shape
    n_edges = h.shape[0]
    npc = n_nodes // NC
    ngrp = (npc + GRP - 1) // GRP

    dst = nbrs[:, 0].astype(np.int64)
    deg = np.bincount(dst, minlength=n_nodes).reshape(NC, npc)
    order = np.argsort(dst, kind="stable")          # edges sorted by dest
    starts = np.zeros(n_nodes + 1, dtype=np.int64)
    np.cumsum(deg.reshape(-1), out=starts[1:])

    # per (core, group): node order by degree desc
    perm = np.zeros((NC, npc), dtype=np.int64)      # processed pos -> local node
    deg_sorted = np.zeros((NC, npc), dtype=np.int64)
    for c in range(NC):
        for g in range(ngrp):
            lo = g * GRP
            hi = min(lo + GRP, npc)
            o = np.argsort(-deg[c, lo:hi], kind="stable")
            perm[c, lo:hi] = lo + o
            deg_sorted[c, lo:hi] = deg[c, lo + o]

    # regularized slot widths K[g][d] = max over cores of #nodes with deg > d
    # (slot 0 forced to full group width so every msg column is initialized)
    K = []
    slot_off = []
    off = 0
    for g in range(ngrp):
        lo = g * GRP
        hi = min(lo + GRP, npc)
        w = hi - lo
        degs = deg_sorted[:, lo:hi]                  # [NC, w]
        dmax = max(int(degs.max()), 1)
        counts = (degs[:, :, None] > np.arange(dmax)[None, None, :]).sum(1)
        Kg = counts.max(0)                           # [dmax]
        Kg[0] = w
        offs = off + np.concatenate([[0], np.cumsum(Kg)])
        K.append(Kg.astype(np.int64))
        slot_off.append(offs.astype(np.int64))
        off = int(offs[-1])
    cols = off

    # col -> edge id (n_edges = zero pad), per core
    col_edge = np.full((NC, cols), n_edges, dtype=np.int64)
    for c in range(NC):
        base = c * npc
        for g in range(ngrp):
            lo = g * GRP
            degs_g = deg_sorted[c, lo:min(lo + GRP, npc)]
            for d in range(len(K[g])):
                kcd = int((degs_g > d).sum())
                if kcd == 0:
                    continue
                nodes = base + perm[c, lo:lo + kcd]
                c0 = slot_off[g][d]
                col_edge[c, c0:c0 + kcd] = order[starts[nodes] + d]

    return {
        "npc": npc, "ngrp": ngrp, "cols": cols, "F": Fdim,
        "K": K, "slot_off": slot_off, "perm": perm, "col_edge": col_edge,
    }


def _build_streams(h, r, lay):
    """Materialize per-core device input arrays."""
    n_edges, Fdim = h.shape
    npc, cols = lay["npc"], lay["cols"]
    fp = Fdim // 128                                 # feature partition-tiles
    sdt = NP_BF16 if STREAM_BF16 else np.float32

    h_aug = np.zeros((n_edges + 1, Fdim), dtype=sdt)
    h_aug[:n_edges] = h.astype(sdt)
    hs, rT = [], []
    for c in range(NC):
        block = h_aug[lay["col_edge"][c]]            # [cols, F]
        hs.append(np.ascontiguousarray(block.T).reshape(fp, 128, cols))
        rc = r[c * npc:(c + 1) * npc][lay["perm"][c]].astype(NP_BF16)
        rT.append(np.ascontiguousarray(rc.T).reshape(fp, 128, npc))
    return hs, rT


# --------------------------------------------------------------- device side
def _pieces_for_group(lay, g):
    """Yield (src_col0, dst_col0, length) spans for group g's slot adds."""
    for d in range(len(lay["K"][g])):
        c0 = int(lay["slot_off"][g][d])
        k = int(lay["K"][g][d])
        yield c0, 0, k


def _build_graph(lay, Fdim, H, Fout):
    npc, ngrp, cols = lay["npc"], lay["ngrp"], lay["cols"]
    fp = Fdim // 128          # 2 feature ptiles
    kt_n = (2 * Fdim) // 128  # 4 k-chunks for W1
    ht_n = H // 128           # 4 hidden ptiles
    ot_n = Fout // 128        # 2 output ptiles
    sdt = BF16 if STREAM_BF16 else F32

    nc = bacc.Bacc(None, target_bir_lowering=False)
    hs_p = nc.declare_dram_parameter("hs", [fp, 128, cols], sdt, isOutput=False)
    rT_p = nc.declare_dram_parameter("rT", [fp, 128, npc], BF16, isOutput=False)
    w1_p = nc.declare_dram_parameter("W1", [kt_n, 128, H], BF16, isOutput=False)
    w2_p = nc.declare_dram_parameter("W2", [ht_n, 128, Fout], BF16, isOutput=False)
    out_p = nc.declare_dram_parameter("out", [ot_n, 128, npc], F32, isOutput=True)

    n_chunks = (cols + CHUNK - 1) // CHUNK

    with tile.TileContext(nc) as tc:
        with (
            tc.tile_pool(name="const", bufs=1) as const_pool,
            tc.tile_pool(name="stream", bufs=4) as stream_pool,
            tc.tile_pool(name="msgp", bufs=2, space="PSUM") as msg_psum_pool,
            tc.tile_pool(name="msgb", bufs=2) as msg_pool,
            tc.tile_pool(name="rb", bufs=2) as r_pool,
            tc.tile_pool(name="mlp1p", bufs=2, space="PSUM") as mlp1_psum_pool,
            tc.tile_pool(name="mlp2p", bufs=2, space="PSUM") as mlp2_psum_pool,
            tc.tile_pool(name="hid", bufs=2) as hid_pool,
            tc.tile_pool(name="osb", bufs=2) as out_pool,
        ):
            # weights resident in SBUF
            w1_sb = []
            for k in range(kt_n):
                t = const_pool.tile([128, H], BF16, tag=f"w1_{k}")
                nc.sync.dma_start(out=t[:], in_=w1_p[k])
                w1_sb.append(t)
            w2_sb = []
            for k in range(ht_n):
                t = const_pool.tile([128, Fout], BF16, tag=f"w2_{k}")
                nc.sync.dma_start(out=t[:], in_=w2_p[k])
                w2_sb.append(t)
            ident = None
            if STREAM_BF16:
                ident = const_pool.tile([128, 128], BF16, tag="ident")
                make_identity(nc, ident)

            chunk_tiles = [[None] * n_chunks for _ in range(fp)]

            def get_chunk(p, ci):
                if chunk_tiles[p][ci] is None:
                    w = min(CHUNK, cols - ci * CHUNK)
                    t = stream_pool.tile([128, w], sdt, tag=f"hs{p}")
                    nc.sync.dma_start(
                        out=t[:], in_=hs_p[p, :, ci * CHUNK:ci * CHUNK + w])
                    chunk_tiles[p][ci] = t
                return chunk_tiles[p][ci]

            for g in range(ngrp):
                lo = g * GRP
                w_g = min(GRP, npc - lo)

                # ---- segment-sum for this group's nodes
                pieces = []
                for c0, d0, k in _pieces_for_group(lay, g):
                    # split on chunk boundaries
                    s = c0
                    while s < c0 + k:
                        ci = s // CHUNK
                        e = min(c0 + k, (ci + 1) * CHUNK)
                        pieces.append((ci, s - ci * CHUNK, d0 + (s - c0), e - s))
                        s = e
                # first-slot pieces initialize (start=True)
                slot0_start = int(lay["slot_off"][g][0])
                slot0_end = slot0_start + int(lay["K"][g][0])

                msgb = []
                for p in range(fp):
                    if STREAM_BF16:
                        ps = msg_psum_pool.tile([128, w_g], F32, space="PSUM",
                                                tag=f"mp{p}")
                        n_p = len(pieces)
                        for i, (ci, o0, dj, ln) in enumerate(pieces):
                            src = get_chunk(p, ci)
                            is_first_slot = (
                                slot0_start <= ci * CHUNK + o0 < slot0_end)
                            nc.tensor.matmul(
                                out=ps[:, dj:dj + ln],
                                lhsT=ident[:],
                                rhs=src[:, o0:o0 + ln],
                                start=is_first_slot,
                                stop=(i == n_p - 1),
                                skip_group_check=True,
                            )
                        mb = msg_pool.tile([128, w_g], BF16, tag=f"mb{p}")
                        nc.scalar.activation(
                            mb[:], ps[:], mybir.ActivationFunctionType.Copy)
                        msgb.append(mb)
                    else:
                        acc = msg_pool.tile([128, w_g], F32, tag=f"macc{p}")
                        nc.any.memset(acc[:], 0.0)
                        for (ci, o0, dj, ln) in pieces:
                            src = get_chunk(p, ci)
                            nc.vector.tensor_tensor(
                                out=acc[:, dj:dj + ln], in0=acc[:, dj:dj + ln],
                                in1=src[:, o0:o0 + ln], op=mybir.AluOpType.add)
                        mb = msg_pool.tile([128, w_g], BF16, tag=f"mb{p}")
                        nc.vector.tensor_copy(out=mb[:], in_=acc[:])
                        msgb.append(mb)

                # ---- r slice (bf16, already permuted on host)
                rb = []
                for p in range(fp):
                    t = r_pool.tile([128, w_g], BF16, tag=f"rb{p}")
                    nc.sync.dma_start(out=t[:], in_=rT_p[p, :, lo:lo + w_g])
                    rb.append(t)
                cat = rb + msgb  # k-chunk order matches W1 rows

                # ---- MLP: hidden^T = relu(W1^T @ cat^T)
                hid = []
                for ht in range(ht_n):
                    ps = mlp1_psum_pool.tile([128, w_g], F32, space="PSUM",
                                             tag="mlp1")
                    for k in range(kt_n):
                        nc.tensor.matmul(
                            out=ps[:],
                            lhsT=w1_sb[k][:, ht * 128:(ht + 1) * 128],
                            rhs=cat[k][:],
                            start=(k == 0), stop=(k == kt_n - 1))
                    hb = hid_pool.tile([128, w_g], BF16, tag=f"h{ht}")
                    nc.scalar.activation(
                        hb[:], ps[:], mybir.ActivationFunctionType.Relu)
                    hid.append(hb)

                # ---- out^T = W2^T @ hidden^T
                for ot in range(ot_n):
                    ps = mlp2_psum_pool.tile([128, w_g], F32, space="PSUM",
                                             tag="mlp2")
                    for k in range(ht_n):
                        nc.tensor.matmul(
                            out=ps[:],
                            lhsT=w2_sb[k][:, ot * 128:(ot + 1) * 128],
                            rhs=hid[k][:],
                            start=(k == 0), stop=(k == ht_n - 1))
                    ob = out_pool.tile([128, w_g], F32, tag=f"o{ot}")
                    nc.vector.tensor_copy(out=ob[:], in_=ps[:])
                    nc.sync.dma_start(out=out_p[ot, :, lo:lo + w_g], in_=ob[:])

    nc.finalize()
    return nc


# ----------------------------------------------------------------- interface
def prepare(r, h, nbrs, W1, W2):
    """Preprocess inputs + build the Bass graph. Returns everything needed
    to run and to assemble the output."""
    r = np.asarray(r, dtype=np.float32)
    h = np.asarray(h, dtype=np.float32)
    nbrs = np.asarray(nbrs)
    W1 = np.asarray(W1, dtype=np.float32)
    W2 = np.asarray(W2, dtype=np.float32)

    n_nodes, Fdim = r.shape
    H = W1.shape[1]
    Fout = W2.shape[1]

    lay = _preprocess(r, h, nbrs)
    hs, rT = _build_streams(h, r, lay)
    w1d = np.ascontiguousarray(W1.astype(NP_BF16)).reshape(-1, 128, H)
    w2d = np.ascontiguousarray(W2.astype(NP_BF16)).reshape(-1, 128, Fout)

    nc = _build_graph(lay, Fdim, H, Fout)
    in_maps = [
        {"hs": hs[c], "rT": rT[c], "W1": w1d, "W2": w2d} for c in range(NC)
    ]
    return {"nc": nc, "in_maps": in_maps, "lay": lay,
            "n_nodes": n_nodes, "Fout": Fout}


def assemble(prep, results):
    lay = prep["lay"]
    n_nodes, Fout = prep["n_nodes"], prep["Fout"]
    npc = lay["npc"]
    out = np.zeros((n_nodes, Fout), dtype=np.float32)
    for c in range(NC):
        o = np.asarray(results[c]["out"]).reshape(Fout, npc)
        sl = slice(c * npc, (c + 1) * npc)
        tmp = np.empty((npc, Fout), dtype=np.float32)
        tmp[lay["perm"][c]] = o.T
        out[sl] = tmp
    return out


def kernel(r, h, nbrs, W1, W2):
    prep = prepare(r, h, nbrs, W1, W2)
    res = run_bass_kernel_spmd(prep["nc"], prep["in_maps"],
                               core_ids=list(range(NC)))
    return assemble(prep, res.results)
